# revision 26
# baseline (speedup 1.0000x reference)
"""Bass/Tile kernel builder for nn_Attention (dense transformer block with
SSF scale-shift, LoRA, parallel adapter, and per-(token,head) router gating),
data-parallel over batch across 8 NeuronCores.

Per-core shapes: x [1024, 768] -> out [1024, 768].
All heavy weights are pre-laid-out on the host (transposed, scales folded):
  - attention scale 1/8 folded into every q-contributing weight
  - ssf_scale1/2 folded into Wqkv / Wproj
  - router index-mixing reshape collapses to: gate(h, j, e) = sig[:, 2*(3h+j)+e]

Layout strategy (no on-device transposes at all):
  - q,k produced in T-layout [ch, n] (lhsT = W tiles, rhs = xT)
  - v produced in natural layout [n, ch] (lhsT = xT tiles, rhs = W)
  - router produced both in T-layout (for PE gate broadcast via 0/1
    selection matmuls) and natural (rides along the v matmul)
  - attention: S^T matmuls, exp on ACT, PV gives O^T directly (v_aug
    carries a ones column so row 64 of PSUM is the softmax denom)

Scheduling strategy (tuned against the timeline cost model; phase 2 is
ACT-bound -- 96 exp tiles at ~1us each -- and every other phase is
PE-bound, so the layout minimizes PE stalls, which cost double on TRN2:
after any idle gap the PE runs at half clock for ~3us of p-state ramp):
  - 1b: the PE only runs matmuls (base/dl/da/gate-select); gated deltas
    and per-channel shifts fold into the PSUM->SBUF move on the DVE
    (scalar_tensor_tensor), with the chain ordered to release the base
    PSUM slot as early as possible.  No PE identity-fold, no ACT bias
    moves.
  - attention pairs are software-pipelined: the PV matmuls of pair p are
    interleaved between the S+exp tiles of pair p+1, so the PE always
    has ready work while ACT drains the exp queue.
  - softmax denominators: raw rowsums copied out as bf16 (DVE), K=1
    bf16 matmuls broadcast them across partitions (the PE waits only on
    the fast DVE copies, never on ACT), inverted via Ln+Exp(-x) on ACT
    behind the exps, multiplied on DVE.  This toolchain rejects
    gpsimd/custom-DVE ISA ops (partition_broadcast, reciprocal_approx)
    and Pool tensor ops, so PE-broadcast + Ln/Exp is the only viable
    scheme.
  - phase 3 lives in the phase-2 scope with its PSUM on the s_ps ring:
    its kk=0..4 projection accumulation only needs pairs 0..4, so the
    PE chews through it while the last pair's Ln/Exp+norm chain drains.
  - proj bias: host-replicated [128,C] tile + DVE add fused into the
    PSUM->SBUF move.
"""
import sys
if "/opt/trn_rl_repo" not in sys.path:
    sys.path.insert(0, "/opt/trn_rl_repo")


import numpy as np
import ml_dtypes

import concourse.bass as bass
import concourse.tile as tile
from concourse import mybir

F32 = mybir.dt.float32
BF16 = mybir.dt.bfloat16
AF = mybir.ActivationFunctionType
ALU = mybir.AluOpType

N, C, H, R, HD = 1024, 768, 12, 64, 64
C3 = 3 * C           # 2304
NT = N // 128        # 8 token tiles
CT = C // 128        # 6 channel tiles
PAIRS = H // 2       # 6 head pairs
NCH = 2              # n-chunks of 512
VW = H * (HD + 1)    # 780: v_aug width (per head: 64 v cols + 1 ones col)


def host_prep(inputs: dict) -> tuple[list[dict], dict]:
    """Fold scales/biases, transpose weights, build per-core in_maps."""
    f = lambda a: np.asarray(a, np.float32)
    x = f(inputs["x"])                      # [8, 1024, 768]
    Wqkv = f(inputs["Wqkv"])                # [2304, 768]
    s1 = f(inputs["ssf_scale1"]); sh1 = f(inputs["ssf_shift1"])
    Wa = f(inputs["Wa"]); Wb = f(inputs["Wb"])
    Wr = f(inputs["Wrouter"]); br = f(inputs["brouter"])
    Wad_d = f(inputs["Wad_down"]); bad_d = f(inputs["bad_down"])
    Wad_u = f(inputs["Wad_up"]); bad_u = f(inputs["bad_up"])
    Wp = f(inputs["Wproj"]); bp = f(inputs["bproj"])
    s2 = f(inputs["ssf_scale2"]); sh2 = f(inputs["ssf_shift2"])

    scale = float(HD) ** -0.5
    W = Wqkv * s1[:, None]
    sh = sh1.copy()
    W[:C] *= scale
    sh[:C] *= scale

    bf = lambda a: np.ascontiguousarray(a, dtype=ml_dtypes.bfloat16)

    wqkT = bf(W[: 2 * C].T)                                   # [768, 1536]
    sh_qk = np.ascontiguousarray(sh[: 2 * C].reshape(12, 128, 1), np.float32)

    # v natural weights + router ride-along, with aug bias row 768
    wv_aug = np.zeros((C + 1, C + 72), np.float32)
    wv_aug[:C, :C] = W[2 * C:].T
    wv_aug[C, :C] = sh[2 * C:]
    wv_aug[:C, C:] = Wr.T
    wv_aug[C, C:] = br
    wv_aug = bf(wv_aug)

    # packed down-projection [768, 128]: cols 0:64 LoRA down, 64:128 adapter
    # down (adapter bias applied via ACT bias on the Gelu)
    wdown = np.zeros((C, 2 * R), np.float32)
    wdown[:, :R] = Wa.T
    wdown[:, R:] = Wad_d.T
    wdown = bf(wdown)
    badd_col = np.ascontiguousarray(bad_d.reshape(R, 1), np.float32)

    # packed up-projection [128, 2304]: rows 0:64 LoRA up (q cols scaled),
    # rows 64:128 adapter up (q cols unused -> 0)
    wup = np.zeros((2 * R, C3), np.float32)
    wup[:R] = Wb.T
    wup[:R, :C] *= scale
    wup[R:, C:] = Wad_u[C:].T
    wup = bf(wup)

    badv_row = bf(bad_u[2 * C:].reshape(1, C))
    badk = np.ascontiguousarray(bad_u[C:2 * C].reshape(6, 128, 1), np.float32)
    brou_col = np.ascontiguousarray(br.reshape(72, 1), np.float32)

    wprojT = np.zeros((C + 1, C), np.float32)
    wprojT[:C] = (Wp * s2[:, None]).T
    wprojT[C] = bp * s2 + sh2
    wprojT = bf(wprojT)

    # selection matrices for gate broadcast: gate row for partition p of
    # q/k ch-tile t (head = 2t + p//64) and member j: sig row 6h + 2j + e
    sel = np.zeros((72, 3 * C), np.float32)
    for si, (j, e) in enumerate([(0, 0), (1, 0), (1, 1)]):   # q_l, k_l, k_p
        for t in range(CT):
            for p in range(128):
                h = 2 * t + p // 64
                sel[6 * h + 2 * j + e, si * C + t * 128 + p] = 1.0
    sel = bf(sel)

    pbias = bf(np.tile((bp * s2 + sh2).reshape(1, C), (128, 1)))

    pairsel = np.zeros((2, 128), np.float32)
    pairsel[0, :64] = 1.0
    pairsel[1, 64:] = 1.0
    pairsel = bf(pairsel)

    shared = dict(wqkT=wqkT, sh_qk=sh_qk, wv_aug=wv_aug, wdown=wdown,
                  badd=badd_col, wup=wup, badv_row=badv_row, badk=badk,
                  brou_col=brou_col, wprojT=wprojT, pbias=pbias, sel=sel,
                  pairsel=pairsel)
    in_maps = []
    for b in range(8):
        m = dict(shared)
        m["xT"] = bf(x[b].T)                                  # [768, 1024]
        in_maps.append(m)
    param_specs = {k: (v.shape, BF16 if v.dtype == ml_dtypes.bfloat16 else F32)
                   for k, v in in_maps[0].items()}
    return in_maps, param_specs


def declare_params(nc, param_specs):
    ext = {}
    for name, (shape, dt) in param_specs.items():
        ext[name] = nc.declare_dram_parameter(name, list(shape), dt, isOutput=False)
    ext["out"] = nc.declare_dram_parameter("out", [N, C], F32, isOutput=True)
    return ext


def build(nc, tc, ctx, ext, iters=1):
    """Emit the kernel body. `iters` repeats compute for steady-state timing."""
    P = lambda name, bufs=1, space=None: ctx.enter_context(
        tc.tile_pool(name=name, bufs=bufs, **({"space": space} if space else {})))

    wpool = P("weights")
    # ---- persistent params -> SBUF ----
    xT = [wpool.tile([128, N], BF16, name=f"xT{t}", tag=f"xT{t}") for t in range(CT)]
    for t in range(CT):
        nc.sync.dma_start(xT[t][:], ext["xT"][t * 128:(t + 1) * 128, :])
    ones_row = wpool.tile([1, N], BF16, name="ones_row", tag="ones_row")
    nc.gpsimd.memset(ones_row[:], 1.0)

    wqk = [wpool.tile([128, 2 * C], BF16, name=f"wqk{t}", tag=f"wqk{t}") for t in range(CT)]
    for t in range(CT):
        nc.sync.dma_start(wqk[t][:], ext["wqkT"][t * 128:(t + 1) * 128, :])
    wv = [wpool.tile([128, C + 72], BF16, name=f"wv{t}", tag=f"wv{t}") for t in range(CT)]
    for t in range(CT):
        nc.sync.dma_start(wv[t][:], ext["wv_aug"][t * 128:(t + 1) * 128, :])
    wv_row = wpool.tile([1, C + 72], BF16, name="wv_row", tag="wv_row")
    nc.sync.dma_start(wv_row[:], ext["wv_aug"][C:C + 1, :])
    wdn = [wpool.tile([128, 2 * R], BF16, name=f"wdn{t}", tag=f"wdn{t}") for t in range(CT)]
    for t in range(CT):
        nc.sync.dma_start(wdn[t][:], ext["wdown"][t * 128:(t + 1) * 128, :])
    badd = wpool.tile([R, 1], F32, name="badd", tag="badd")
    nc.sync.dma_start(badd[:], ext["badd"][:, :])
    wup = wpool.tile([128, C3], BF16, name="wup", tag="wup")
    nc.sync.dma_start(wup[:], ext["wup"][:, :])
    badv = wpool.tile([1, C], BF16, name="badv", tag="badv")
    nc.sync.dma_start(badv[:], ext["badv_row"][:, :])
    wpj = [wpool.tile([128, C], BF16, name=f"wpj{t}", tag=f"wpj{t}") for t in range(CT)]
    for t in range(CT):
        nc.sync.dma_start(wpj[t][:], ext["wprojT"][t * 128:(t + 1) * 128, :])
    pbias = wpool.tile([128, C], BF16, name="pbias", tag="pbias")
    nc.sync.dma_start(pbias[:], ext["pbias"][:, :])
    sel = wpool.tile([72, 3 * C], BF16, name="sel", tag="sel")
    nc.sync.dma_start(sel[:], ext["sel"][:, :])
    psel_a = wpool.tile([1, 128], BF16, name="psel_a", tag="psel_a")
    nc.sync.dma_start(psel_a[:], ext["pairsel"][0:1, :])
    psel_b = wpool.tile([1, 128], BF16, name="psel_b", tag="psel_b")
    nc.sync.dma_start(psel_b[:], ext["pairsel"][1:2, :])
    sh_qk = [wpool.tile([128, 1], F32, name=f"shqk{t}", tag=f"shqk{t}") for t in range(12)]
    for t in range(12):
        nc.sync.dma_start(sh_qk[t][:], ext["sh_qk"][t])
    badk = [wpool.tile([128, 1], F32, name=f"badk{t}", tag=f"badk{t}") for t in range(CT)]
    for t in range(CT):
        nc.sync.dma_start(badk[t][:], ext["badk"][t])
    brou = wpool.tile([72, 1], F32, name="brou", tag="brou")
    nc.sync.dma_start(brou[:], ext["brou_col"][:, :])

    kw = dict(xT=xT, ones_row=ones_row, wqk=wqk, wv=wv, wv_row=wv_row,
              wdn=wdn, badd=badd, wup=wup, badv=badv, wpj=wpj, pbias=pbias,
              sel=sel, psel_a=psel_a, psel_b=psel_b, sh_qk=sh_qk, badk=badk, brou=brou)
    if iters == 1:
        _one_iter(nc, tc, ctx, ext, 0, **kw)
    else:
        # timing mode: run the body `iters` times inside one NEFF
        with tc.For_i(0, iters, 1):
            _one_iter(nc, tc, ctx, ext, 0, **kw)


def _one_iter(nc, tc, ctx, ext, it, *, xT, ones_row, wqk, wv, wv_row, wdn,
              badd, wup, badv, wpj, pbias, sel, psel_a, psel_b, sh_qk, badk, brou):
    from contextlib import ExitStack
    P = lambda name, bufs=1, space=None, c=None: (c or ctx).enter_context(
        tc.tile_pool(name=name, bufs=bufs, **({"space": space} if space else {})))

    big = P(f"big{it}")
    qT = [big.tile([128, N], BF16, name=f"qT{t}", tag=f"qT{t}") for t in range(CT)]
    kT = [big.tile([128, N], BF16, name=f"kT{t}", tag=f"kT{t}") for t in range(CT)]
    vaug = [big.tile([128, VW], BF16, name=f"vaug{t}", tag=f"vaug{t}") for t in range(NT)]
    # oT aliases the qT tiles: qT[p] is last read by S(p), and the
    # normalized O^T of pair p is written strictly after that, so reusing
    # the buffers saves 12KB of SBUF (the tile framework orders the
    # write-after-read automatically).
    oT = qT
    rT_sig = big.tile([72, N], BF16, name="rT_sig", tag="rT_sig")
    rnat = [big.tile([128, 72], F32, name=f"rnat{t}", tag=f"rnat{t}") for t in range(NT)]
    down = big.tile([128, N], BF16, name="down", tag="down")

    # ======== phase 1a: down-projection (LoRA + adapter) and router-T ======
    with ExitStack() as ph:
        pp = P("ph1a_psum", 1, "PSUM", ph)
        dn_ps = pp.tile([128, N], F32, name="dn_ps", tag="dn_ps")
        for c in range(NCH):
            s = slice(c * 512, (c + 1) * 512)
            for t in range(CT):
                nc.tensor.matmul(dn_ps[:, s], wdn[t][:], xT[t][:, s],
                                 start=(t == 0), stop=(t == CT - 1))
        nc.scalar.activation(down[0:64, :], dn_ps[0:64, :], AF.Copy)
        nc.scalar.activation(down[64:128, :], dn_ps[64:128, :], AF.Gelu,
                             bias=badd[:])

        rt_ps = pp.tile([72, N], F32, name="rt_ps", tag="rt_ps")
        for c in range(NCH):
            s = slice(c * 512, (c + 1) * 512)
            for t in range(CT):
                nc.tensor.matmul(rt_ps[:, s], wv[t][:, C:C + 72], xT[t][:, s],
                                 start=(t == 0), stop=(t == CT - 1))
        nc.scalar.activation(rT_sig[:], rt_ps[:], AF.Sigmoid, bias=brou[:])

    # ======== phase 1b: q,k in T-layout with gating =========================
    # The PE only runs matmuls (base/dl/da/gates); the gated delta + shift
    # are folded into the PSUM->SBUF move on the DVE so the PE never stalls
    # (a stall also drops the PE to half clock for ~3us).
    with ExitStack() as ph:
        ps_base = P("qk_base", 2, "PSUM", ph)   # 2 banks
        ps_dl = P("qk_dl", 2, "PSUM", ph)       # 2 banks
        ps_da = P("qk_da", 2, "PSUM", ph)       # 2 banks
        ps_g = P("qk_g", 2, "PSUM", ph)         # 2 banks
        tmp = P("qk_tmp", 2, c=ph)
        for t in range(12):  # 0..5 q-tiles, 6..11 k-tiles
            is_q = t < 6
            ct6 = t % 6
            for c in range(NCH):
                s = slice(c * 512, (c + 1) * 512)
                base = ps_base.tile([128, 512], F32, name="base", tag="base")
                for kk in range(CT):
                    nc.tensor.matmul(base[:], wqk[kk][:, t * 128:(t + 1) * 128],
                                     xT[kk][:, s], start=(kk == 0),
                                     stop=(kk == CT - 1))
                dl = ps_dl.tile([128, 512], F32, name="dl", tag="dl")
                nc.tensor.matmul(dl[:], wup[0:64, t * 128:(t + 1) * 128],
                                 down[0:64, s], start=True, stop=True)
                # base+shift leaves PSUM via ACT (fast, frees the slot
                # early); the delta algebra then runs on the DVE in pure
                # bf16 SBUF, which gets the 4x perf mode.
                btmp = tmp.tile([128, 512], BF16, name="btmp", tag="btmp")
                nc.scalar.activation(btmp[:], base[:], AF.Identity,
                                     bias=sh_qk[t][:])
                if is_q:
                    g = ps_g.tile([128, 512], F32, name="g", tag="g")
                    nc.tensor.matmul(g[:], sel[:, ct6 * 128:(ct6 + 1) * 128],
                                     rT_sig[:, s], start=True, stop=True)
                    g_sb = tmp.tile([128, 512], BF16, name="g_sb", tag="g_sb")
                    nc.scalar.activation(g_sb[:], g[:], AF.Copy)
                    tq = tmp.tile([128, 512], BF16, name="tq", tag="tq")
                    nc.vector.tensor_tensor(tq[:], dl[:], g_sb[:], ALU.mult)
                    nc.vector.tensor_tensor(qT[ct6][:, s], btmp[:], tq[:],
                                            ALU.add)
                else:
                    da = ps_da.tile([128, 512], F32, name="da", tag="da")
                    nc.tensor.matmul(da[:], wup[64:128, t * 128:(t + 1) * 128],
                                     down[64:128, s], start=True, stop=True)
                    gl = ps_g.tile([128, 512], F32, name="g", tag="g")
                    nc.tensor.matmul(gl[:],
                                     sel[:, C + ct6 * 128:C + (ct6 + 1) * 128],
                                     rT_sig[:, s], start=True, stop=True)
                    gp = ps_g.tile([128, 512], F32, name="g", tag="g")
                    nc.tensor.matmul(gp[:],
                                     sel[:, 2 * C + ct6 * 128:2 * C + (ct6 + 1) * 128],
                                     rT_sig[:, s], start=True, stop=True)
                    gl_sb = tmp.tile([128, 512], BF16, name="g_sb", tag="g_sb")
                    nc.scalar.activation(gl_sb[:], gl[:], AF.Copy)
                    gp_sb = tmp.tile([128, 512], BF16, name="gp_sb", tag="gp_sb")
                    nc.scalar.activation(gp_sb[:], gp[:], AF.Copy)
                    t1 = tmp.tile([128, 512], BF16, name="t1", tag="t1")
                    nc.vector.tensor_tensor(t1[:], dl[:], gl_sb[:], ALU.mult)
                    t2 = tmp.tile([128, 512], BF16, name="t2", tag="t2")
                    nc.vector.scalar_tensor_tensor(
                        t2[:], da[:], badk[ct6][:], gp_sb[:],
                        ALU.add, ALU.mult)
                    s12 = tmp.tile([128, 512], BF16, name="s12", tag="s12")
                    nc.vector.tensor_tensor(s12[:], t1[:], t2[:], ALU.add)
                    nc.vector.tensor_tensor(kT[ct6][:, s], btmp[:], s12[:],
                                            ALU.add)

    # ======== phase 1c: v natural with gating + router-natural =============
    with ExitStack() as ph:
        ps_v = P("v_base", 2, "PSUM", ph)   # [128,840]=2 banks x2 = 4
        ps_vd = P("v_dl", 1, "PSUM", ph)    # dlv 2 + dav 2 = 4 banks
        tmp = P("v_tmp", 2, c=ph)
        for nt in range(NT):
            ns = slice(nt * 128, (nt + 1) * 128)
            vb = ps_v.tile([128, C + 72], F32, name="vb", tag="vb")
            for (off, sz) in ((0, 512), (512, C + 72 - 512)):
                o = slice(off, off + sz)
                for kk in range(CT):
                    nc.tensor.matmul(vb[:, o], xT[kk][:, ns], wv[kk][:, o],
                                     start=(kk == 0), stop=False)
                nc.tensor.matmul(vb[:, o], ones_row[:, ns], wv_row[:, o],
                                 start=False, stop=True)
            nc.scalar.activation(rnat[nt][:], vb[:, C:C + 72], AF.Sigmoid)
            dlv = ps_vd.tile([128, C], F32, name="dlv", tag="dlv")
            dav = ps_vd.tile([128, C], F32, name="dav", tag="dav")
            for (off, sz) in ((0, 512), (512, 256)):
                o = slice(off, off + sz)
                vo = slice(2 * C + off, 2 * C + off + sz)
                nc.tensor.matmul(dlv[:, o], down[0:64, ns], wup[0:64, vo],
                                 start=True, stop=True)
                nc.tensor.matmul(dav[:, o], down[64:128, ns], wup[64:128, vo],
                                 start=True, stop=False)
                nc.tensor.matmul(dav[:, o], ones_row[:, ns], badv[:, o],
                                 start=False, stop=True)
            gl = rnat[nt][:, 4:72:6].unsqueeze(2).to_broadcast((128, 12, 64))
            gp = rnat[nt][:, 5:72:6].unsqueeze(2).to_broadcast((128, 12, 64))
            t1 = tmp.tile([128, C], BF16, name="vt1", tag="vt1")
            nc.vector.tensor_tensor(
                t1[:].rearrange("p (h d) -> p h d", h=12),
                dlv[:].rearrange("p (h d) -> p h d", h=12), gl, ALU.mult)
            t2 = tmp.tile([128, C], BF16, name="vt2", tag="vt2")
            nc.vector.tensor_tensor(
                t2[:].rearrange("p (h d) -> p h d", h=12),
                dav[:].rearrange("p (h d) -> p h d", h=12), gp, ALU.mult)
            t3 = tmp.tile([128, C], BF16, name="vt3", tag="vt3")
            nc.vector.tensor_tensor(t3[:], t1[:], t2[:], ALU.add)
            nc.gpsimd.memset(vaug[nt][:, 64:VW:65], 1.0)
            vout = vaug[nt][:, 0:VW].rearrange("p (h x) -> p h x", h=12)[:, :, 0:64]
            nc.vector.tensor_tensor(
                vout, vb[:, 0:C].rearrange("p (h d) -> p h d", h=12),
                t3[:].rearrange("p (h d) -> p h d", h=12), ALU.add)

    # ======== phase 2: attention, software-pipelined across head pairs =====
    # Emission order S(0) | S(1), PV(0) | S(2), PV(1) | ... keeps the PE's
    # in-order queue busy with pair p+1's score matmuls while ACT
    # exponentiates pair p; PV(p) then finds every E tile ready.
    with ExitStack() as ph:
        epool = P("att_e", 34, c=ph)
        rpool = P("att_recip", 2, c=ph)
        bpool = P("att_bc", 2, c=ph)
        ps_s = P("att_s", 2, "PSUM", ph)
        ps_o = P("att_o", 2, "PSUM", ph)
        Es = {}

        def s_tile(p, mt, hi, hh):
            s_ps = ps_s.tile([128, 1024], F32, name="s_ps", tag="s_ps")
            for c in range(NCH):
                nc.tensor.matmul(
                    s_ps[:, c * 512:(c + 1) * 512],
                    kT[p][hh:hh + 64, mt * 128:(mt + 1) * 128],
                    qT[p][hh:hh + 64, c * 512:(c + 1) * 512],
                    start=True, stop=True)
            e = epool.tile([128, 1024], BF16, name="e", tag="e")
            nc.scalar.activation(e[:], s_ps[:], AF.Exp)
            Es[(p, mt, hi)] = e

        def s_exp(p, interleave=None):
            """Emit the 16 S+exp tiles of pair p; between tiles, emit the
            interleaved PV matmuls of the previous pair so the PE always
            has ready work while ACT drains the exp queue (a bare wait
            would also knock the PE down to half clock for ~3us)."""
            units = list(interleave) if interleave else []
            ui = 0
            step = max(1, (len(units) + 15) // 16)
            for i, (mt, (hi, hh)) in enumerate(
                    (m, h) for m in range(NT) for h in ((0, 0), (1, 64))):
                s_tile(p, mt, hi, hh)
                take = units[ui:ui + step]
                ui += step
                for emit in take:
                    emit()
            for emit in units[ui:]:
                emit()

        def pv_units(p, o_ps):
            """Return emission thunks for PV(p)'s 32 matmuls (accumulation
            groups interleave across PSUM banks; group check skipped)."""
            units = []
            for hi, hh in ((0, 0), (1, 64)):
                h = 2 * p + hi
                for c in range(NCH):
                    for mt in range(NT):
                        def emit(hi=hi, c=c, mt=mt, h=h):
                            nc.tensor.matmul(
                                o_ps[hi][:, c * 512:(c + 1) * 512],
                                vaug[mt][:, h * 65:h * 65 + 65],
                                Es[(p, mt, hi)][:, c * 512:(c + 1) * 512],
                                start=(mt == 0), stop=(mt == NT - 1),
                                skip_group_check=True)
                        units.append(emit)
            return units

        def pv_norm(p, o_ps, bpool_p, btag):
            for mt in range(NT):
                for hi in (0, 1):
                    del Es[(p, mt, hi)]
            # softmax denominators: 1/rowsum via fast-approx reciprocal on
            # the DVE (18 bits, denominators are O(100..3000) so edge cases
            # are unreachable), partition-broadcast on Pool, multiply fused
            # into the PSUM->SBUF move on DVE.
            # denominators: copy the raw rowsum rows out as bf16 (DVE),
            # broadcast across partitions with two K=1 bf16 matmuls (the
            # PE only waits on the fast DVE copies, never on ACT), then
            # invert via Ln + Exp(-x) on ACT (queued behind the exps, off
            # the PE's critical path), multiply on DVE.
            rc = [rpool.tile([1, N], BF16, name="rc", tag=f"rc{hi}")
                  for hi in (0, 1)]
            for hi in (0, 1):
                nc.vector.tensor_copy(rc[hi][:], o_ps[hi][64:65, :])
            b_ps = bpool_p.tile([128, N], F32, name="b_ps", tag=btag)
            for c in range(NCH):
                s = slice(c * 512, (c + 1) * 512)
                nc.tensor.matmul(b_ps[:, s], psel_a[:], rc[0][:, s],
                                 start=True, stop=False)
                nc.tensor.matmul(b_ps[:, s], psel_b[:], rc[1][:, s],
                                 start=False, stop=True)
            b_sb = bpool.tile([128, N], BF16, name="b_sb", tag="b_sb")
            nc.scalar.activation(b_sb[:], b_ps[:], AF.Ln)
            nc.scalar.activation(b_sb[:], b_sb[:], AF.Exp, scale=-1.0)
            for hi, hh in ((0, 0), (1, 64)):
                nc.vector.tensor_tensor(oT[p][hh:hh + 64, :],
                                        o_ps[hi][0:64, :],
                                        b_sb[hh:hh + 64, :], ALU.mult)

        opool = P("proj_sb", 4, c=ph)
        s_exp(0)
        for p in range(PAIRS):
            # the last pair's accumulators ride the s_ps ring: its slots
            # free right after the final exps, while the o_ps ring would
            # chain PV(5) behind pair 4's whole Ln/Exp+norm tail.
            pool_p, tag_p = (ps_s, "s_ps") if p == PAIRS - 1 else (ps_o, "o_ps")
            o_ps = [pool_p.tile([65, N], F32, name="o_ps", tag=tag_p)
                    for _ in range(2)]
            units = pv_units(p, o_ps)
            if p + 1 < PAIRS:
                s_exp(p + 1, interleave=units)
            else:
                for emit in units:
                    emit()
            pv_norm(p, o_ps, *((ps_o, "o_ps") if p == PAIRS - 1
                               else (ps_s, "s_ps")))

        # ---- phase 3 (same scope): projection + output -------------------
        # po rides the s_ps PSUM ring (same slot width).  The kk=0..4
        # accumulation only needs pairs 0..4 normalized, so the PE chews
        # through it while the last pair's Ln/Exp+norm chain drains.
        for nt in range(NT):
            ns = slice(nt * 128, (nt + 1) * 128)
            po = (ps_s if nt % 2 == 0 else ps_o).tile(
                [128, 1024], F32, name="po",
                tag="s_ps" if nt % 2 == 0 else "o_ps")
            for (off, sz) in ((0, 512), (512, 256)):
                o = slice(off, off + sz)
                for kk in range(CT):
                    nc.tensor.matmul(po[:, o], oT[kk][:, ns], wpj[kk][:, o],
                                     start=(kk == 0), stop=(kk == CT - 1))
            osb = opool.tile([128, C], F32, name="osb", tag="osb")
            nc.vector.tensor_tensor(osb[:], po[:, 0:C], pbias[:], ALU.add)
            nc.sync.dma_start(ext["out"][ns, :], osb[:])


# ---------------------------------------------------------------------------
# walrus workaround: this build rejects instructions carrying more than one
# sync-wait command; split excess waits onto same-engine NoOps placed just
# before the over-subscribed instruction (engines are in-order, so waiting
# earlier on the same engine is equivalent).

def split_sync_waits(nc, cap=1):
    for fn in nc.m.functions:
        for bb in fn.blocks:
            new_insts = []
            changed = False
            for inst in bb.instructions:
                si = inst.sync_info
                waits = list(si.on_wait) if si is not None else []
                if len(waits) > cap:
                    changed = True
                    extra, keep = waits[:-cap], waits[-cap:]
                    while extra:
                        chunk, extra = extra[:cap], extra[cap:]
                        nop = mybir.InstNoOp(
                            name=f"I-waitsplit-{nc.next_id()}",
                            engine=inst.engine,
                            bass_nofuse=True,
                            sync_info=mybir.SyncInfo(on_wait=chunk,
                                                     on_update=[]),
                        )
                        new_insts.append(nop)
                    si.on_wait.clear()
                    for w in keep:
                        si.on_wait.append(w)
                    inst.sync_info = si
                new_insts.append(inst)
            if changed:
                bb.instructions = new_insts


# ---------------------------------------------------------------------------
_CACHE = {}


def _get_nc(iters=1):
    if iters not in _CACHE:
        from contextlib import ExitStack
        nc = bass.Bass("TRN2", target_bir_lowering=False, debug=False,
                       num_devices=8)
        specs = {
            "wqkT": ((C, 2 * C), BF16),
            "sh_qk": ((12, 128, 1), F32),
            "wv_aug": ((C + 1, C + 72), BF16),
            "wdown": ((C, 2 * R), BF16),
            "badd": ((R, 1), F32),
            "wup": ((2 * R, C3), BF16),
            "badv_row": ((1, C), BF16),
            "badk": ((6, 128, 1), F32),
            "brou_col": ((72, 1), F32),
            "wprojT": ((C + 1, C), BF16),
            "pbias": ((128, C), BF16),
            "pairsel": ((2, 128), BF16),
            "sel": ((72, 3 * C), BF16),
            "xT": ((C, N), BF16),
        }
        ext = declare_params(nc, specs)
        with tile.TileContext(nc) as tc:
            with ExitStack() as ctx:
                build(nc, tc, ctx, ext, iters=iters)
        split_sync_waits(nc)
        _CACHE[iters] = nc
    return _CACHE[iters]


def kernel(**inputs):
    """Full-input, full-output entry point.

    Shards data-parallel over batch across the 8 NeuronCores (weights
    replicated, pre-transposed/folded on host), runs the Bass kernel via
    run_bass_kernel_spmd, and stacks the per-core outputs.
    """
    from concourse.bass_utils import run_bass_kernel_spmd
    in_maps, _ = host_prep(inputs)
    nc = _get_nc(iters=1)
    res = run_bass_kernel_spmd(nc, in_maps, core_ids=list(range(8)))
    out = np.stack([res.results[i]["out"] for i in range(8)], axis=0)
    return out.astype(np.float32)


# revision 27
# speedup vs baseline: 1.2119x; 1.2119x over previous
"""Bass/Tile kernel builder for nn_Attention (dense transformer block with
SSF scale-shift, LoRA, parallel adapter, and per-(token,head) router gating),
data-parallel over batch across 8 NeuronCores.

Per-core shapes: x [1024, 768] -> out [1024, 768].
All heavy weights are pre-laid-out on the host (transposed, scales folded):
  - attention scale 1/8 folded into every q-contributing weight
  - ssf_scale1/2 folded into Wqkv / Wproj
  - router index-mixing reshape collapses to: gate(h, j, e) = sig[:, 2*(3h+j)+e]

Layout strategy (no on-device transposes at all):
  - q,k produced in T-layout [ch, n] (lhsT = W tiles, rhs = xT)
  - v produced in natural layout [n, ch] (lhsT = xT tiles, rhs = W)
  - router produced both in T-layout (for PE gate broadcast via 0/1
    selection matmuls) and natural (rides along the v matmul)
  - attention: S^T matmuls, exp on ACT, PV gives O^T directly (v_aug
    carries a ones column so row 64 of PSUM is the softmax denom)

Scheduling strategy (tuned against the timeline cost model; phase 2 is
ACT-bound -- 96 exp tiles at ~1us each -- and every other phase is
PE-bound, so the layout minimizes PE stalls, which cost double on TRN2:
after any idle gap the PE runs at half clock for ~3us of p-state ramp):
  - 1b: the PE only runs matmuls (base/dl/da/gate-select); gated deltas
    and per-channel shifts fold into the PSUM->SBUF move on the DVE
    (scalar_tensor_tensor), with the chain ordered to release the base
    PSUM slot as early as possible.  No PE identity-fold, no ACT bias
    moves.
  - attention pairs are software-pipelined: the PV matmuls of pair p are
    interleaved between the S+exp tiles of pair p+1, so the PE always
    has ready work while ACT drains the exp queue.
  - softmax denominators: raw rowsums copied out as bf16 (DVE), K=1
    bf16 matmuls broadcast them across partitions (the PE waits only on
    the fast DVE copies, never on ACT), inverted via Ln+Exp(-x) on ACT
    behind the exps, multiplied on DVE.  This toolchain rejects
    gpsimd/custom-DVE ISA ops (partition_broadcast, reciprocal_approx)
    and Pool tensor ops, so PE-broadcast + Ln/Exp is the only viable
    scheme.
  - phase 3 lives in the phase-2 scope with its PSUM on the s_ps ring:
    its kk=0..4 projection accumulation only needs pairs 0..4, so the
    PE chews through it while the last pair's Ln/Exp+norm chain drains.
  - proj bias: host-replicated [128,C] tile + DVE add fused into the
    PSUM->SBUF move.
"""
import sys
if "/opt/trn_rl_repo" not in sys.path:
    sys.path.insert(0, "/opt/trn_rl_repo")


import numpy as np
import ml_dtypes

import concourse.bass as bass
import concourse.tile as tile
from concourse import mybir

F32 = mybir.dt.float32
BF16 = mybir.dt.bfloat16
AF = mybir.ActivationFunctionType
ALU = mybir.AluOpType

N, C, H, R, HD = 1024, 768, 12, 64, 64
C3 = 3 * C           # 2304
NT = N // 128        # 8 token tiles
CT = C // 128        # 6 channel tiles
PAIRS = H // 2       # 6 head pairs
NCH = 2              # n-chunks of 512
VW = H * (HD + 1)    # 780: v_aug width (per head: 64 v cols + 1 ones col)


def host_prep(inputs: dict) -> tuple[list[dict], dict]:
    """Fold scales/biases, transpose weights, build per-core in_maps."""
    f = lambda a: np.asarray(a, np.float32)
    x = f(inputs["x"])                      # [8, 1024, 768]
    Wqkv = f(inputs["Wqkv"])                # [2304, 768]
    s1 = f(inputs["ssf_scale1"]); sh1 = f(inputs["ssf_shift1"])
    Wa = f(inputs["Wa"]); Wb = f(inputs["Wb"])
    Wr = f(inputs["Wrouter"]); br = f(inputs["brouter"])
    Wad_d = f(inputs["Wad_down"]); bad_d = f(inputs["bad_down"])
    Wad_u = f(inputs["Wad_up"]); bad_u = f(inputs["bad_up"])
    Wp = f(inputs["Wproj"]); bp = f(inputs["bproj"])
    s2 = f(inputs["ssf_scale2"]); sh2 = f(inputs["ssf_shift2"])

    scale = float(HD) ** -0.5
    W = Wqkv * s1[:, None]
    sh = sh1.copy()
    W[:C] *= scale
    sh[:C] *= scale

    bf = lambda a: np.ascontiguousarray(a, dtype=ml_dtypes.bfloat16)

    wqkT = bf(W[: 2 * C].T)                                   # [768, 1536]
    sh_qk = np.ascontiguousarray(sh[: 2 * C].reshape(12, 128, 1), np.float32)

    # v natural weights + router ride-along, with aug bias row 768
    wv_aug = np.zeros((C + 1, C + 72), np.float32)
    wv_aug[:C, :C] = W[2 * C:].T
    wv_aug[C, :C] = sh[2 * C:]
    wv_aug[:C, C:] = Wr.T
    wv_aug[C, C:] = br
    wv_aug = bf(wv_aug)

    # packed down-projection [768, 128]: cols 0:64 LoRA down, 64:128 adapter
    # down (adapter bias applied via ACT bias on the Gelu)
    wdown = np.zeros((C, 2 * R), np.float32)
    wdown[:, :R] = Wa.T
    wdown[:, R:] = Wad_d.T
    wdown = bf(wdown)
    badd_col = np.ascontiguousarray(bad_d.reshape(R, 1), np.float32)

    # packed up-projection [128, 2304]: rows 0:64 LoRA up (q cols scaled),
    # rows 64:128 adapter up (q cols unused -> 0)
    wup = np.zeros((2 * R, C3), np.float32)
    wup[:R] = Wb.T
    wup[:R, :C] *= scale
    wup[R:, C:] = Wad_u[C:].T
    wup = bf(wup)

    badv_row = bf(bad_u[2 * C:].reshape(1, C))
    badk = np.ascontiguousarray(bad_u[C:2 * C].reshape(6, 128, 1), np.float32)
    brou_col = np.ascontiguousarray(br.reshape(72, 1), np.float32)

    wprojT = np.zeros((C + 1, C), np.float32)
    wprojT[:C] = (Wp * s2[:, None]).T
    wprojT[C] = bp * s2 + sh2
    wprojT = bf(wprojT)

    # selection matrices for gate broadcast: gate row for partition p of
    # q/k ch-tile t (head = 2t + p//64) and member j: sig row 6h + 2j + e
    sel = np.zeros((72, 3 * C), np.float32)
    for si, (j, e) in enumerate([(0, 0), (1, 0), (1, 1)]):   # q_l, k_l, k_p
        for t in range(CT):
            for p in range(128):
                h = 2 * t + p // 64
                sel[6 * h + 2 * j + e, si * C + t * 128 + p] = 1.0
    sel = bf(sel)

    pbias = bf(np.tile((bp * s2 + sh2).reshape(1, C), (128, 1)))

    pairsel = np.zeros((2, 128), np.float32)
    pairsel[0, :64] = 1.0
    pairsel[1, 64:] = 1.0
    pairsel = bf(pairsel)

    shared = dict(wqkT=wqkT, sh_qk=sh_qk, wv_aug=wv_aug, wdown=wdown,
                  badd=badd_col, wup=wup, badv_row=badv_row, badk=badk,
                  brou_col=brou_col, wprojT=wprojT, pbias=pbias, sel=sel,
                  pairsel=pairsel)
    in_maps = []
    for b in range(8):
        m = dict(shared)
        m["xT"] = bf(x[b].T)                                  # [768, 1024]
        in_maps.append(m)
    param_specs = {k: (v.shape, BF16 if v.dtype == ml_dtypes.bfloat16 else F32)
                   for k, v in in_maps[0].items()}
    return in_maps, param_specs


def declare_params(nc, param_specs):
    ext = {}
    for name, (shape, dt) in param_specs.items():
        ext[name] = nc.declare_dram_parameter(name, list(shape), dt, isOutput=False)
    ext["out"] = nc.declare_dram_parameter("out", [N, C], F32, isOutput=True)
    return ext


def build(nc, tc, ctx, ext, iters=1):
    """Emit the kernel body. `iters` repeats compute for steady-state timing."""
    P = lambda name, bufs=1, space=None: ctx.enter_context(
        tc.tile_pool(name=name, bufs=bufs, **({"space": space} if space else {})))

    wpool = P("weights")
    # ---- persistent params -> SBUF ----
    xT = [wpool.tile([128, N], BF16, name=f"xT{t}", tag=f"xT{t}") for t in range(CT)]
    for t in range(CT):
        nc.sync.dma_start(xT[t][:], ext["xT"][t * 128:(t + 1) * 128, :])
    ones_row = wpool.tile([1, N], BF16, name="ones_row", tag="ones_row")
    nc.gpsimd.memset(ones_row[:], 1.0)

    wqk = [wpool.tile([128, 2 * C], BF16, name=f"wqk{t}", tag=f"wqk{t}") for t in range(CT)]
    for t in range(CT):
        nc.sync.dma_start(wqk[t][:], ext["wqkT"][t * 128:(t + 1) * 128, :])
    wv = [wpool.tile([128, C + 72], BF16, name=f"wv{t}", tag=f"wv{t}") for t in range(CT)]
    for t in range(CT):
        nc.sync.dma_start(wv[t][:], ext["wv_aug"][t * 128:(t + 1) * 128, :])
    wv_row = wpool.tile([1, C + 72], BF16, name="wv_row", tag="wv_row")
    nc.sync.dma_start(wv_row[:], ext["wv_aug"][C:C + 1, :])
    wdn = [wpool.tile([128, 2 * R], BF16, name=f"wdn{t}", tag=f"wdn{t}") for t in range(CT)]
    for t in range(CT):
        nc.sync.dma_start(wdn[t][:], ext["wdown"][t * 128:(t + 1) * 128, :])
    badd = wpool.tile([R, 1], F32, name="badd", tag="badd")
    nc.sync.dma_start(badd[:], ext["badd"][:, :])
    wup = wpool.tile([128, C3], BF16, name="wup", tag="wup")
    nc.sync.dma_start(wup[:], ext["wup"][:, :])
    badv = wpool.tile([1, C], BF16, name="badv", tag="badv")
    nc.sync.dma_start(badv[:], ext["badv_row"][:, :])
    wpj = [wpool.tile([128, C], BF16, name=f"wpj{t}", tag=f"wpj{t}") for t in range(CT)]
    for t in range(CT):
        nc.sync.dma_start(wpj[t][:], ext["wprojT"][t * 128:(t + 1) * 128, :])
    pbias = wpool.tile([128, C], BF16, name="pbias", tag="pbias")
    nc.sync.dma_start(pbias[:], ext["pbias"][:, :])
    sel = wpool.tile([72, 3 * C], BF16, name="sel", tag="sel")
    nc.sync.dma_start(sel[:], ext["sel"][:, :])
    psel_a = wpool.tile([1, 128], BF16, name="psel_a", tag="psel_a")
    nc.sync.dma_start(psel_a[:], ext["pairsel"][0:1, :])
    psel_b = wpool.tile([1, 128], BF16, name="psel_b", tag="psel_b")
    nc.sync.dma_start(psel_b[:], ext["pairsel"][1:2, :])
    sh_qk = [wpool.tile([128, 1], F32, name=f"shqk{t}", tag=f"shqk{t}") for t in range(12)]
    for t in range(12):
        nc.sync.dma_start(sh_qk[t][:], ext["sh_qk"][t])
    badk = [wpool.tile([128, 1], F32, name=f"badk{t}", tag=f"badk{t}") for t in range(CT)]
    for t in range(CT):
        nc.sync.dma_start(badk[t][:], ext["badk"][t])
    brou = wpool.tile([72, 1], F32, name="brou", tag="brou")
    nc.sync.dma_start(brou[:], ext["brou_col"][:, :])

    kw = dict(xT=xT, ones_row=ones_row, wqk=wqk, wv=wv, wv_row=wv_row,
              wdn=wdn, badd=badd, wup=wup, badv=badv, wpj=wpj, pbias=pbias,
              sel=sel, psel_a=psel_a, psel_b=psel_b, sh_qk=sh_qk, badk=badk, brou=brou)
    if iters == 1:
        _one_iter(nc, tc, ctx, ext, 0, **kw)
    else:
        # timing mode: run the body `iters` times inside one NEFF
        with tc.For_i(0, iters, 1):
            _one_iter(nc, tc, ctx, ext, 0, **kw)


def _one_iter(nc, tc, ctx, ext, it, *, xT, ones_row, wqk, wv, wv_row, wdn,
              badd, wup, badv, wpj, pbias, sel, psel_a, psel_b, sh_qk, badk, brou):
    from contextlib import ExitStack
    P = lambda name, bufs=1, space=None, c=None: (c or ctx).enter_context(
        tc.tile_pool(name=name, bufs=bufs, **({"space": space} if space else {})))

    big = P(f"big{it}")
    qT = [big.tile([128, N], BF16, name=f"qT{t}", tag=f"qT{t}") for t in range(CT)]
    kT = [big.tile([128, N], BF16, name=f"kT{t}", tag=f"kT{t}") for t in range(CT)]
    vaug = [big.tile([128, VW], BF16, name=f"vaug{t}", tag=f"vaug{t}") for t in range(NT)]
    # oT aliases the qT tiles: qT[p] is last read by S(p), and the
    # normalized O^T of pair p is written strictly after that, so reusing
    # the buffers saves 12KB of SBUF (the tile framework orders the
    # write-after-read automatically).
    oT = qT
    rT_sig = big.tile([72, N], BF16, name="rT_sig", tag="rT_sig")
    rnat = [big.tile([128, 72], F32, name=f"rnat{t}", tag=f"rnat{t}") for t in range(NT)]
    down = big.tile([128, N], BF16, name="down", tag="down")

    # ======== phase 1a: down-projection (LoRA + adapter) and router-T ======
    with ExitStack() as ph:
        pp = P("ph1a_psum", 1, "PSUM", ph)
        dn_ps = pp.tile([128, N], F32, name="dn_ps", tag="dn_ps")
        for c in range(NCH):
            s = slice(c * 512, (c + 1) * 512)
            for t in range(CT):
                nc.tensor.matmul(dn_ps[:, s], wdn[t][:], xT[t][:, s],
                                 start=(t == 0), stop=(t == CT - 1))
        nc.scalar.activation(down[0:64, :], dn_ps[0:64, :], AF.Copy)
        nc.scalar.activation(down[64:128, :], dn_ps[64:128, :], AF.Gelu,
                             bias=badd[:])

        rt_ps = pp.tile([72, N], F32, name="rt_ps", tag="rt_ps")
        for c in range(NCH):
            s = slice(c * 512, (c + 1) * 512)
            for t in range(CT):
                nc.tensor.matmul(rt_ps[:, s], wv[t][:, C:C + 72], xT[t][:, s],
                                 start=(t == 0), stop=(t == CT - 1))
        nc.scalar.activation(rT_sig[:], rt_ps[:], AF.Sigmoid, bias=brou[:])

    # ======== phase 1b: q,k in T-layout with gating =========================
    # The PE only runs matmuls (base/dl/da/gates); the gated delta + shift
    # are folded into the PSUM->SBUF move on the DVE so the PE never stalls
    # (a stall also drops the PE to half clock for ~3us).
    with ExitStack() as ph:
        ps_base = P("qk_base", 2, "PSUM", ph)   # 2 banks
        ps_dl = P("qk_dl", 2, "PSUM", ph)       # 2 banks
        ps_da = P("qk_da", 2, "PSUM", ph)       # 2 banks
        ps_g = P("qk_g", 2, "PSUM", ph)         # 2 banks
        tmp = P("qk_tmp", 2, c=ph)
        for t in range(12):  # 0..5 q-tiles, 6..11 k-tiles
            is_q = t < 6
            ct6 = t % 6
            for c in range(NCH):
                s = slice(c * 512, (c + 1) * 512)
                base = ps_base.tile([128, 512], F32, name="base", tag="base")
                for kk in range(CT):
                    nc.tensor.matmul(base[:], wqk[kk][:, t * 128:(t + 1) * 128],
                                     xT[kk][:, s], start=(kk == 0),
                                     stop=(kk == CT - 1))
                dl = ps_dl.tile([128, 512], F32, name="dl", tag="dl")
                nc.tensor.matmul(dl[:], wup[0:64, t * 128:(t + 1) * 128],
                                 down[0:64, s], start=True, stop=True)
                # base+shift leaves PSUM via ACT (fast, frees the slot
                # early); the delta algebra then runs on the DVE in pure
                # bf16 SBUF, which gets the 4x perf mode.
                btmp = tmp.tile([128, 512], BF16, name="btmp", tag="btmp")
                nc.scalar.activation(btmp[:], base[:], AF.Identity,
                                     bias=sh_qk[t][:])
                if is_q:
                    g = ps_g.tile([128, 512], F32, name="g", tag="g")
                    nc.tensor.matmul(g[:], sel[:, ct6 * 128:(ct6 + 1) * 128],
                                     rT_sig[:, s], start=True, stop=True)
                    g_sb = tmp.tile([128, 512], BF16, name="g_sb", tag="g_sb")
                    nc.scalar.activation(g_sb[:], g[:], AF.Copy)
                    tq = tmp.tile([128, 512], BF16, name="tq", tag="tq")
                    nc.vector.tensor_tensor(tq[:], dl[:], g_sb[:], ALU.mult)
                    nc.vector.tensor_tensor(qT[ct6][:, s], btmp[:], tq[:],
                                            ALU.add)
                else:
                    da = ps_da.tile([128, 512], F32, name="da", tag="da")
                    nc.tensor.matmul(da[:], wup[64:128, t * 128:(t + 1) * 128],
                                     down[64:128, s], start=True, stop=True)
                    gl = ps_g.tile([128, 512], F32, name="g", tag="g")
                    nc.tensor.matmul(gl[:],
                                     sel[:, C + ct6 * 128:C + (ct6 + 1) * 128],
                                     rT_sig[:, s], start=True, stop=True)
                    gp = ps_g.tile([128, 512], F32, name="g", tag="g")
                    nc.tensor.matmul(gp[:],
                                     sel[:, 2 * C + ct6 * 128:2 * C + (ct6 + 1) * 128],
                                     rT_sig[:, s], start=True, stop=True)
                    gl_sb = tmp.tile([128, 512], BF16, name="g_sb", tag="g_sb")
                    nc.scalar.activation(gl_sb[:], gl[:], AF.Copy)
                    gp_sb = tmp.tile([128, 512], BF16, name="gp_sb", tag="gp_sb")
                    nc.scalar.activation(gp_sb[:], gp[:], AF.Copy)
                    t1 = tmp.tile([128, 512], BF16, name="t1", tag="t1")
                    nc.vector.tensor_tensor(t1[:], dl[:], gl_sb[:], ALU.mult)
                    t2 = tmp.tile([128, 512], BF16, name="t2", tag="t2")
                    nc.vector.scalar_tensor_tensor(
                        t2[:], da[:], badk[ct6][:], gp_sb[:],
                        ALU.add, ALU.mult)
                    s12 = tmp.tile([128, 512], BF16, name="s12", tag="s12")
                    nc.vector.tensor_tensor(s12[:], t1[:], t2[:], ALU.add)
                    nc.vector.tensor_tensor(kT[ct6][:, s], btmp[:], s12[:],
                                            ALU.add)

    # ======== phase 1c: v natural with gating + router-natural =============
    with ExitStack() as ph:
        ps_v = P("v_base", 2, "PSUM", ph)   # [128,840]=2 banks x2 = 4
        ps_vd = P("v_dl", 1, "PSUM", ph)    # dlv 2 + dav 2 = 4 banks
        tmp = P("v_tmp", 2, c=ph)
        for nt in range(NT):
            ns = slice(nt * 128, (nt + 1) * 128)
            vb = ps_v.tile([128, C + 72], F32, name="vb", tag="vb")
            for (off, sz) in ((0, 512), (512, C + 72 - 512)):
                o = slice(off, off + sz)
                for kk in range(CT):
                    nc.tensor.matmul(vb[:, o], xT[kk][:, ns], wv[kk][:, o],
                                     start=(kk == 0), stop=False)
                nc.tensor.matmul(vb[:, o], ones_row[:, ns], wv_row[:, o],
                                 start=False, stop=True)
            nc.scalar.activation(rnat[nt][:], vb[:, C:C + 72], AF.Sigmoid)
            dlv = ps_vd.tile([128, C], F32, name="dlv", tag="dlv")
            dav = ps_vd.tile([128, C], F32, name="dav", tag="dav")
            for (off, sz) in ((0, 512), (512, 256)):
                o = slice(off, off + sz)
                vo = slice(2 * C + off, 2 * C + off + sz)
                nc.tensor.matmul(dlv[:, o], down[0:64, ns], wup[0:64, vo],
                                 start=True, stop=True)
                nc.tensor.matmul(dav[:, o], down[64:128, ns], wup[64:128, vo],
                                 start=True, stop=False)
                nc.tensor.matmul(dav[:, o], ones_row[:, ns], badv[:, o],
                                 start=False, stop=True)
            gl = rnat[nt][:, 4:72:6].unsqueeze(2).to_broadcast((128, 12, 64))
            gp = rnat[nt][:, 5:72:6].unsqueeze(2).to_broadcast((128, 12, 64))
            t1 = tmp.tile([128, C], BF16, name="vt1", tag="vt1")
            nc.vector.tensor_tensor(
                t1[:].rearrange("p (h d) -> p h d", h=12),
                dlv[:].rearrange("p (h d) -> p h d", h=12), gl, ALU.mult)
            t2 = tmp.tile([128, C], BF16, name="vt2", tag="vt2")
            nc.vector.tensor_tensor(
                t2[:].rearrange("p (h d) -> p h d", h=12),
                dav[:].rearrange("p (h d) -> p h d", h=12), gp, ALU.mult)
            t3 = tmp.tile([128, C], BF16, name="vt3", tag="vt3")
            nc.vector.tensor_tensor(t3[:], t1[:], t2[:], ALU.add)
            nc.gpsimd.memset(vaug[nt][:, 64:VW:65], 1.0)
            vout = vaug[nt][:, 0:VW].rearrange("p (h x) -> p h x", h=12)[:, :, 0:64]
            nc.vector.tensor_tensor(
                vout, vb[:, 0:C].rearrange("p (h d) -> p h d", h=12),
                t3[:].rearrange("p (h d) -> p h d", h=12), ALU.add)

    # ======== phase 2: attention, software-pipelined across head pairs =====
    # Emission order S(0) | S(1), PV(0) | S(2), PV(1) | ... keeps the PE's
    # in-order queue busy with pair p+1's score matmuls while ACT
    # exponentiates pair p; PV(p) then finds every E tile ready.
    with ExitStack() as ph:
        epool = P("att_e", 34, c=ph)
        rpool = P("att_recip", 2, c=ph)
        bpool = P("att_bc", 2, c=ph)
        ps_s = P("att_s", 2, "PSUM", ph)
        ps_o = P("att_o", 2, "PSUM", ph)
        Es = {}

        def s_tile(p, mt, hi, hh):
            s_ps = ps_s.tile([128, 1024], F32, name="s_ps", tag="s_ps")
            for c in range(NCH):
                nc.tensor.matmul(
                    s_ps[:, c * 512:(c + 1) * 512],
                    kT[p][hh:hh + 64, mt * 128:(mt + 1) * 128],
                    qT[p][hh:hh + 64, c * 512:(c + 1) * 512],
                    start=True, stop=True)
            e = epool.tile([128, 1024], BF16, name="e", tag="e")
            nc.scalar.activation(e[:], s_ps[:], AF.Exp)
            Es[(p, mt, hi)] = e

        def s_exp(p, interleave=None):
            """Emit the 16 S+exp tiles of pair p; between tiles, emit the
            interleaved PV matmuls of the previous pair so the PE always
            has ready work while ACT drains the exp queue (a bare wait
            would also knock the PE down to half clock for ~3us)."""
            units = list(interleave) if interleave else []
            ui = 0
            step = max(1, (len(units) + 15) // 16)
            for i, (mt, (hi, hh)) in enumerate(
                    (m, h) for m in range(NT) for h in ((0, 0), (1, 64))):
                s_tile(p, mt, hi, hh)
                take = units[ui:ui + step]
                ui += step
                for emit in take:
                    emit()
            for emit in units[ui:]:
                emit()

        def pv_units(p, o_ps):
            """Return emission thunks for PV(p)'s 32 matmuls (accumulation
            groups interleave across PSUM banks; group check skipped)."""
            units = []
            for hi, hh in ((0, 0), (1, 64)):
                h = 2 * p + hi
                for c in range(NCH):
                    for mt in range(NT):
                        def emit(hi=hi, c=c, mt=mt, h=h):
                            nc.tensor.matmul(
                                o_ps[hi][:, c * 512:(c + 1) * 512],
                                vaug[mt][:, h * 65:h * 65 + 65],
                                Es[(p, mt, hi)][:, c * 512:(c + 1) * 512],
                                start=(mt == 0), stop=(mt == NT - 1),
                                skip_group_check=True)
                        units.append(emit)
            return units

        def pv_norm(p, o_ps):
            for mt in range(NT):
                for hi in (0, 1):
                    del Es[(p, mt, hi)]
            # softmax denominators: 1/rowsum via fast-approx reciprocal on
            # the DVE (18 bits, denominators are O(100..3000) so edge cases
            # are unreachable), partition-broadcast on Pool, multiply fused
            # into the PSUM->SBUF move on DVE.
            # denominators: copy the raw rowsum rows out as bf16 (DVE),
            # broadcast across partitions with two K=1 bf16 matmuls (the
            # PE only waits on the fast DVE copies, never on ACT), then
            # invert via Ln + Exp(-x) on ACT (queued behind the exps, off
            # the PE's critical path), multiply on DVE.
            rc = [rpool.tile([1, N], BF16, name="rc", tag=f"rc{hi}")
                  for hi in (0, 1)]
            for hi in (0, 1):
                nc.vector.tensor_copy(rc[hi][:], o_ps[hi][64:65, :])
            b_ps = ps_s.tile([128, N], F32, name="b_ps", tag="s_ps")
            for c in range(NCH):
                s = slice(c * 512, (c + 1) * 512)
                nc.tensor.matmul(b_ps[:, s], psel_a[:], rc[0][:, s],
                                 start=True, stop=False)
                nc.tensor.matmul(b_ps[:, s], psel_b[:], rc[1][:, s],
                                 start=False, stop=True)
            b_sb = bpool.tile([128, N], BF16, name="b_sb", tag="b_sb")
            nc.scalar.activation(b_sb[:], b_ps[:], AF.Ln)
            nc.scalar.activation(b_sb[:], b_sb[:], AF.Exp, scale=-1.0)
            for hi, hh in ((0, 0), (1, 64)):
                nc.vector.tensor_tensor(oT[p][hh:hh + 64, :],
                                        o_ps[hi][0:64, :],
                                        b_sb[hh:hh + 64, :], ALU.mult)

        opool = P("proj_sb", 4, c=ph)
        s_exp(0)
        for p in range(PAIRS):
            o_ps = [ps_o.tile([65, N], F32, name="o_ps", tag="o_ps")
                    for _ in range(2)]
            units = pv_units(p, o_ps)
            if p + 1 < PAIRS:
                s_exp(p + 1, interleave=units)
            else:
                for emit in units:
                    emit()
            pv_norm(p, o_ps)

        # ---- phase 3 (same scope): projection + output -------------------
        # po rides the s_ps PSUM ring (same slot width).  The kk=0..4
        # accumulation only needs pairs 0..4 normalized, so the PE chews
        # through it while the last pair's Ln/Exp+norm chain drains.
        for nt in range(NT):
            ns = slice(nt * 128, (nt + 1) * 128)
            po = (ps_s if nt % 2 == 0 else ps_o).tile(
                [128, 1024], F32, name="po",
                tag="s_ps" if nt % 2 == 0 else "o_ps")
            for (off, sz) in ((0, 512), (512, 256)):
                o = slice(off, off + sz)
                for kk in range(CT):
                    nc.tensor.matmul(po[:, o], oT[kk][:, ns], wpj[kk][:, o],
                                     start=(kk == 0), stop=(kk == CT - 1))
            osb = opool.tile([128, C], F32, name="osb", tag="osb")
            nc.vector.tensor_tensor(osb[:], po[:, 0:C], pbias[:], ALU.add)
            nc.sync.dma_start(ext["out"][ns, :], osb[:])


# ---------------------------------------------------------------------------
# walrus workaround: this build rejects instructions carrying more than one
# sync-wait command; split excess waits onto same-engine NoOps placed just
# before the over-subscribed instruction (engines are in-order, so waiting
# earlier on the same engine is equivalent).

def split_sync_waits(nc, cap=1):
    for fn in nc.m.functions:
        for bb in fn.blocks:
            new_insts = []
            changed = False
            for inst in bb.instructions:
                si = inst.sync_info
                waits = list(si.on_wait) if si is not None else []
                if len(waits) > cap:
                    changed = True
                    extra, keep = waits[:-cap], waits[-cap:]
                    while extra:
                        chunk, extra = extra[:cap], extra[cap:]
                        nop = mybir.InstNoOp(
                            name=f"I-waitsplit-{nc.next_id()}",
                            engine=inst.engine,
                            bass_nofuse=True,
                            sync_info=mybir.SyncInfo(on_wait=chunk,
                                                     on_update=[]),
                        )
                        new_insts.append(nop)
                    si.on_wait.clear()
                    for w in keep:
                        si.on_wait.append(w)
                    inst.sync_info = si
                new_insts.append(inst)
            if changed:
                bb.instructions = new_insts


# ---------------------------------------------------------------------------
_CACHE = {}


def _get_nc(iters=1):
    if iters not in _CACHE:
        from contextlib import ExitStack
        nc = bass.Bass("TRN2", target_bir_lowering=False, debug=False,
                       num_devices=8)
        specs = {
            "wqkT": ((C, 2 * C), BF16),
            "sh_qk": ((12, 128, 1), F32),
            "wv_aug": ((C + 1, C + 72), BF16),
            "wdown": ((C, 2 * R), BF16),
            "badd": ((R, 1), F32),
            "wup": ((2 * R, C3), BF16),
            "badv_row": ((1, C), BF16),
            "badk": ((6, 128, 1), F32),
            "brou_col": ((72, 1), F32),
            "wprojT": ((C + 1, C), BF16),
            "pbias": ((128, C), BF16),
            "pairsel": ((2, 128), BF16),
            "sel": ((72, 3 * C), BF16),
            "xT": ((C, N), BF16),
        }
        ext = declare_params(nc, specs)
        with tile.TileContext(nc) as tc:
            with ExitStack() as ctx:
                build(nc, tc, ctx, ext, iters=iters)
        split_sync_waits(nc)
        _CACHE[iters] = nc
    return _CACHE[iters]


def kernel(**inputs):
    """Full-input, full-output entry point.

    Shards data-parallel over batch across the 8 NeuronCores (weights
    replicated, pre-transposed/folded on host), runs the Bass kernel via
    run_bass_kernel_spmd, and stacks the per-core outputs.
    """
    from concourse.bass_utils import run_bass_kernel_spmd
    in_maps, _ = host_prep(inputs)
    nc = _get_nc(iters=1)
    res = run_bass_kernel_spmd(nc, in_maps, core_ids=list(range(8)))
    out = np.stack([res.results[i]["out"] for i in range(8)], axis=0)
    return out.astype(np.float32)
